# revision 1
# baseline (speedup 1.0000x reference)
"""GAT 2-layer kernel for Trainium2, 8 NeuronCores.

Sharding: nodes are relabeled host-side so each core owns a contiguous block
of 12544 node ids with balanced edge counts (degree-sorted round-robin deal,
then an in-core sort by max-per-bank edge count to minimize gather padding).
Edges are partitioned by src row, so the per-row softmax and the output
accumulation are core-local.  Each core computes v = tanh(x@Wv.T+bv) and the
per-node scalar s = tanh(x@Wa.T+ba)@We.T for its own nodes into a [128]-wide
table row (v | s | pad), an AllGather shares the table, and the edge stage
fetches one 256B table row per edge with dma_gather (InstDMAGatherAnt).

dma_gather indices are int16, so the 100352-row table is gathered in 4 banks
of 25088 rows; each node's edge slots are grouped by dst bank with a uniform
per-group depth D (slot space [bank][tile][j]).  Pad slots gather row 0 of
the bank and are zeroed by a host-built mask after exp.

Key algebraic simplification: (a[src]+a[dst]) @ We.T = s[src] + s[dst] with
s = a@We.T, so the edge MLP needs only per-node scalars.
"""

import numpy as np
from contextlib import ExitStack

import concourse.bass as bass
import concourse.tile as tile
from concourse import bacc, mybir
from concourse.bass_utils import run_bass_kernel_spmd

F32 = mybir.dt.float32
I16 = mybir.dt.int16
BF16 = mybir.dt.bfloat16
FP16 = mybir.dt.float16

NCORES = 8
N = 100_000
E = 1_600_000
D0 = 128
D1 = 64
NBANK = 4
ROWW = 128          # table row width (elements)
ZDT = FP16          # table dtype; 256B rows
S_MAX = 128
G_MAX = 8


# ---------------------------------------------------------------------------
# host-side planning (pure index manipulation)
# ---------------------------------------------------------------------------

def build_plan(edge_index, n=N, ncores=NCORES):
    src = np.asarray(edge_index[0], dtype=np.int64)
    dst = np.asarray(edge_index[1], dtype=np.int64)
    deg = np.bincount(src, minlength=n).astype(np.int64)

    ranks_per_core = (n + ncores - 1) // ncores           # 12500
    tiles = (ranks_per_core + 127) // 128                  # 98
    npc = tiles * 128                                      # 12544
    npad = ncores * npc                                    # 100352
    bs = npad // NBANK                                     # 25088 (= 2 cores)

    # pass 1: deal nodes to cores by degree rank (balances edges per core);
    # the core then determines each node's bank (= core // 2).
    order = np.argsort(-deg, kind="stable")
    core_of_old = np.empty(n, dtype=np.int64)
    core_of_old[order] = np.arange(n) % ncores

    # per-node, per-bank out-edge counts (bank of an edge = dst's core // 2)
    bank_of_old_dst = core_of_old[dst] // 2
    cnt = np.zeros((n, NBANK), dtype=np.int64)
    np.add.at(cnt, (src, bank_of_old_dst), 1)
    maxb = cnt.max(1)

    # pass 2: within each core, order nodes by max-per-bank count desc
    new_of_old = np.empty(n, dtype=np.int64)
    for c in range(ncores):
        own = np.where(core_of_old == c)[0]
        own = own[np.argsort(-maxb[own], kind="stable")]
        new_of_old[own] = c * npc + np.arange(len(own))

    nsrc = new_of_old[src]
    ndst = new_of_old[dst]
    # group edges by (src, bank)
    ekey = nsrc * NBANK + (ndst // bs)
    eorder = np.argsort(ekey, kind="stable")
    ndst_sorted = ndst[eorder]
    ecnt = np.bincount(ekey, minlength=npad * NBANK).reshape(npad, NBANK)
    estart = np.zeros(npad * NBANK + 1, dtype=np.int64)
    np.cumsum(ecnt.reshape(-1), out=estart[1:])
    estart = estart[:-1].reshape(npad, NBANK)

    # per-tile depth D (max over cores, banks, nodes in tile), aligned to 4
    cnt_new = np.zeros((npad, NBANK), dtype=np.int64)
    cnt_new[new_of_old] = cnt
    # pad nodes (deg 0): one fake edge in bank 0 so softmax denom > 0
    padnode = cnt_new.sum(1) == 0
    cnt_new[padnode, 0] = 1
    c4 = cnt_new.reshape(ncores, tiles, 128, NBANK)
    dt_tile = c4.max(axis=(0, 2, 3))                      # [tiles]
    dt_tile = np.maximum(((dt_tile + 3) // 4) * 4, 4)

    # group consecutive tiles sharing padded depth, S = NBANK*G*D <= S_MAX
    groups = []
    t = 0
    while t < tiles:
        d = int(dt_tile[t])
        g = 1
        while (t + g < tiles and g < G_MAX
               and NBANK * (g + 1) * d <= S_MAX
               and dt_tile[t + g] <= d):
            g += 1
        groups.append((t, g, d))
        t += g

    s_total = sum(NBANK * g * d for (_, g, d) in groups)
    idxcols = sum(NBANK * 8 * g * d for (_, g, d) in groups)

    gidx = np.zeros((ncores, 128, idxcols), dtype=np.int16)
    maskh = np.zeros((ncores, 128, s_total), dtype=np.float32)
    jj64 = np.arange(64, dtype=np.int64)

    for ci in range(ncores):
        soff = 0
        ioff = 0
        for (t0, G, D) in groups:
            ni = 128 * G * D
            nids = (ci * npc + t0 * 128 + np.arange(G * 128)).reshape(G, 128)
            for k in range(NBANK):
                st = estart[nids, k][:, :, None] + jj64[None, None, :D]
                cn = cnt_new[nids, k]
                valid = jj64[None, None, :D] < cn[:, :, None]
                vals = np.where(valid, ndst_sorted[np.minimum(st, E - 1)] - k * bs, 0)
                # fake edge for pad nodes: bank 0, idx 0 (valid already set)
                if k == 0:
                    pd = padnode[nids]
                    valid = valid | (pd[:, :, None] & (jj64[None, None, :D] < 1))
                    vals = np.where(pd[:, :, None], 0, vals)
                # position i = (g*D + j)*128 + p  ->  idx16[16q + i%16, i//16]
                pos_val = vals.transpose(1, 0, 2).reshape(128, G * D)  # [p, g*D+j]
                flat = np.empty(ni, np.int16)
                gdj = np.arange(G * D)
                # i = gdj*128 + p
                flat = pos_val.T.reshape(-1)                 # [(g j) p] -> i order
                wrapped = np.zeros((16, ni // 16), np.int16)
                ii = np.arange(ni)
                wrapped[ii % 16, ii // 16] = flat
                gidx[ci, :, ioff:ioff + ni // 16] = np.tile(wrapped, (8, 1))
                mk = valid.transpose(1, 0, 2).reshape(128, G * D).astype(np.float32)
                maskh[ci, :, soff + k * G * D: soff + (k + 1) * G * D] = mk
                ioff += ni // 16
            soff += NBANK * G * D

    return dict(
        new_of_old=new_of_old, npc=npc, npad=npad, tiles=tiles, bs=bs,
        groups=groups, s_total=s_total, idxcols=idxcols, gidx=gidx,
        maskh=maskh, ranks_per_core=ranks_per_core,
    )


def host_inputs(h, plan, W11, b11, W12, b12, W13, b13,
                W21, b21, W22, b22, W23, b23):
    npc, ncores = plan["npc"], NCORES
    new_of_old = plan["new_of_old"]
    h_new = np.zeros((plan["npad"], D0), dtype=np.float32)
    h_new[new_of_old] = np.asarray(h, dtype=np.float32)

    rep = np.ones((128, 1), dtype=np.float32)
    base = dict(
        waT1=np.ascontiguousarray(W11.T.astype(np.float32)),
        wvT1=np.ascontiguousarray(W13.T.astype(np.float32)),
        ba1r=rep * b11[None, :].astype(np.float32),
        bv1r=rep * b13[None, :].astype(np.float32),
        we1r=rep * W12[0][None, :].astype(np.float32),
        wa2e=np.concatenate([W21.T, b21[None, :]], 0).astype(np.float32),
        wv2e=np.concatenate([W23.T, b23[None, :]], 0).astype(np.float32),
        we2r=rep * W22[0][None, :].astype(np.float32),
        ident=np.eye(128, dtype=np.float32),
    )
    be1_half = float(b12[0]) * 0.5
    be2_half = float(b22[0]) * 0.5

    in_maps = []
    for c in range(ncores):
        m = dict(base)
        m["hT"] = np.ascontiguousarray(h_new[c * npc:(c + 1) * npc].T)
        m["gidx"] = np.ascontiguousarray(plan["gidx"][c])
        m["maskh"] = np.ascontiguousarray(plan["maskh"][c])
        in_maps.append(m)
    return in_maps, be1_half, be2_half


# ---------------------------------------------------------------------------
# device program (single SPMD program; per-core differences live in the data)
# ---------------------------------------------------------------------------

def build_device_program(plan, be1_half, be2_half, ncores=NCORES, stage=4):
    npc = plan["npc"]
    npad = plan["npad"]
    tiles = plan["tiles"]
    groups = plan["groups"]
    s_total = plan["s_total"]
    idxcols = plan["idxcols"]
    bs = plan["bs"]

    nc = bacc.Bacc("TRN2", target_bir_lowering=False, debug=False,
                   num_devices=ncores, dynamic_dma_scratch_size=32768)

    hT = nc.dram_tensor("hT", [D0, npc], F32, kind="ExternalInput")
    gidx_d = nc.dram_tensor("gidx", [128, idxcols], I16, kind="ExternalInput")
    mask_d = nc.dram_tensor("maskh", [128, s_total], F32, kind="ExternalInput")
    waT1 = nc.dram_tensor("waT1", [D0, D1], F32, kind="ExternalInput")
    wvT1 = nc.dram_tensor("wvT1", [D0, D1], F32, kind="ExternalInput")
    ba1r = nc.dram_tensor("ba1r", [128, D1], F32, kind="ExternalInput")
    bv1r = nc.dram_tensor("bv1r", [128, D1], F32, kind="ExternalInput")
    we1r = nc.dram_tensor("we1r", [128, D1], F32, kind="ExternalInput")
    wa2e = nc.dram_tensor("wa2e", [D1 + 1, D1], F32, kind="ExternalInput")
    wv2e = nc.dram_tensor("wv2e", [D1 + 1, D1], F32, kind="ExternalInput")
    we2r = nc.dram_tensor("we2r", [128, D1], F32, kind="ExternalInput")
    ident_d = nc.dram_tensor("ident", [128, 128], F32, kind="ExternalInput")
    out_my = nc.dram_tensor("out", [npc, D1], F32, kind="ExternalOutput")

    z1my = nc.dram_tensor("z1my", [npc, ROWW], ZDT)
    z2my = nc.dram_tensor("z2my", [npc, ROWW], ZDT)
    z1 = nc.dram_tensor("z1", [npad, ROWW], ZDT, addr_space="Shared")
    z2 = nc.dram_tensor("z2", [npad, ROWW], ZDT, addr_space="Shared")
    replica_groups = [list(range(ncores))]

    with tile.TileContext(nc) as tc, ExitStack() as ctx:
        const_p = ctx.enter_context(tc.tile_pool(name="const", bufs=1))
        xt_p = ctx.enter_context(tc.tile_pool(name="xt", bufs=2))
        ps_p = ctx.enter_context(tc.tile_pool(name="ps", bufs=2, space="PSUM"))
        pst_p = ctx.enter_context(tc.tile_pool(name="pst", bufs=2, space="PSUM"))
        dense_p = ctx.enter_context(tc.tile_pool(name="dense", bufs=2))
        zrow_p = ctx.enter_context(tc.tile_pool(name="zrow", bufs=2))
        idx_p = ctx.enter_context(tc.tile_pool(name="idx", bufs=2))
        zg_p = ctx.enter_context(tc.tile_pool(name="zg", bufs=2))
        w_p = ctx.enter_context(tc.tile_pool(name="w", bufs=1))
        ed_p = ctx.enter_context(tc.tile_pool(name="ed", bufs=2))
        sm_p = ctx.enter_context(tc.tile_pool(name="sm", bufs=1))

        def ld_const(dram, shape, tag, dt=F32):
            t = const_p.tile(shape, dt, tag=tag)
            nc.sync.dma_start(t[:], dram[:])
            return t

        waT1_sb = ld_const(waT1, [D0, D1], "waT1")
        wvT1_sb = ld_const(wvT1, [D0, D1], "wvT1")
        ba1_sb = ld_const(ba1r, [128, D1], "ba1")
        bv1_sb = ld_const(bv1r, [128, D1], "bv1")
        we1_sb = ld_const(we1r, [128, D1], "we1")
        wa2_sb = ld_const(wa2e, [D1 + 1, D1], "wa2")
        wv2_sb = ld_const(wv2e, [D1 + 1, D1], "wv2")
        we2_sb = ld_const(we2r, [128, D1], "we2")
        mask_sb = ld_const(mask_d, [128, s_total], "mask")
        ident = ld_const(ident_d, [128, 128], "ident")

        s_own1 = const_p.tile([128, tiles], F32)
        s_own2 = const_p.tile([128, tiles], F32)
        h1T_sb = const_p.tile([D1 + 1, npc], F32)
        nc.vector.memset(h1T_sb[D1:D1 + 1, :], 1.0)

        # ----------------- dense stage -----------------
        def dense_layer(layer, zmy, s_own):
            for bt in range(0, tiles, 8):
                nb = min(8, tiles - bt)
                zrow = zrow_p.tile([128, 8 * ROWW], ZDT, tag="zrow")
                nc.vector.memset(zrow[:], 0)
                zr = zrow[:].rearrange("p (k c) -> p k c", c=ROWW)
                for k in range(nb):
                    t = bt + k
                    if layer == 1:
                        xt = xt_p.tile([D0, 128], F32)
                        nc.sync.dma_start(xt[:], hT[:, t * 128:(t + 1) * 128])
                        lhs, wa, wv = xt[:], waT1_sb[:], wvT1_sb[:]
                    else:
                        lhs, wa, wv = (h1T_sb[:, t * 128:(t + 1) * 128],
                                       wa2_sb[:], wv2_sb[:])
                    pa = ps_p.tile([128, D1], F32, tag="pa")
                    nc.tensor.matmul(pa[:], lhsT=lhs, rhs=wa, start=True, stop=True)
                    pv = ps_p.tile([128, D1], F32, tag="pv")
                    nc.tensor.matmul(pv[:], lhsT=lhs, rhs=wv, start=True, stop=True)
                    a_sb = dense_p.tile([128, D1], F32, tag="a")
                    if layer == 1:
                        apre = dense_p.tile([128, D1], F32, tag="apre")
                        nc.vector.tensor_add(apre[:], pa[:], ba1_sb[:])
                        nc.scalar.activation(a_sb[:], apre[:],
                                             mybir.ActivationFunctionType.Tanh)
                        vpre = dense_p.tile([128, D1], F32, tag="vpre")
                        nc.vector.tensor_add(vpre[:], pv[:], bv1_sb[:])
                        nc.scalar.activation(zr[:, k, 0:64], vpre[:],
                                             mybir.ActivationFunctionType.Tanh)
                    else:
                        nc.scalar.activation(a_sb[:], pa[:],
                                             mybir.ActivationFunctionType.Tanh)
                        nc.scalar.activation(zr[:, k, 0:64], pv[:],
                                             mybir.ActivationFunctionType.Tanh)
                    scr = dense_p.tile([128, D1], F32, tag="scr")
                    nc.vector.tensor_mul(
                        scr[:], a_sb[:],
                        we1_sb[:] if layer == 1 else we2_sb[:])
                    rs = dense_p.tile([128, 1], F32, tag="rs")
                    nc.vector.reduce_sum(out=rs[:], in_=scr[:],
                                         axis=mybir.AxisListType.X)
                    nc.vector.tensor_scalar_add(
                        s_own[:, t:t + 1], rs[:],
                        be1_half if layer == 1 else be2_half)
                    nc.vector.tensor_copy(zr[:, k, 64:65], s_own[:, t:t + 1])
                dr = zmy[:].rearrange("(t p) c -> p t c", p=128)
                nc.sync.dma_start(dr[:, bt:bt + nb, :], zr[:, 0:nb, :])

        # ----------------- edge stage -----------------
        def edge_layer(z_full, s_own, layer):
            soff = 0
            ioff = 0
            for (t0, G, D) in groups:
                GD = G * D
                S = NBANK * GD
                ni = 128 * GD
                it = idx_p.tile([128, NBANK * (ni // 16)], I16, tag="it")
                nc.sync.dma_start(it[:], gidx_d[:, ioff:ioff + NBANK * (ni // 16)])
                zg = zg_p.tile([128, S * ROWW], ZDT, tag="zg")
                # the SWDGE descriptor ring holds 2048 descs; chunk gathers
                # to 8 slots (1024 descs) so two can be in flight
                CH = 8
                for k in range(NBANK):
                    for j0 in range(0, GD, CH):
                        js = min(CH, GD - j0)
                        nj = 128 * js
                        nc.gpsimd.dma_gather(
                            out_ap=zg[:, (k * GD + j0) * ROWW:
                                      (k * GD + j0 + js) * ROWW]
                            .rearrange("p (s c) -> p s c", c=ROWW),
                            in_ap=z_full[k * bs:(k + 1) * bs, :],
                            idxs_ap=it[:, k * (ni // 16) + j0 * 8:
                                       k * (ni // 16) + (j0 + js) * 8],
                            num_idxs=nj, num_idxs_reg=nj, elem_size=ROWW)
                zg3 = zg[:].rearrange("p (s c) -> p s c", c=ROWW)

                epre = ed_p.tile([128, S], F32, tag="epre")
                sown_b = (s_own[:, t0:t0 + G].unsqueeze(1)
                          .broadcast_to([128, NBANK, G]).unsqueeze(3)
                          .broadcast_to([128, NBANK, G, D]))
                nc.vector.tensor_tensor(
                    out=epre[:].rearrange("p (k g j) -> p k g j", k=NBANK, g=G),
                    in0=zg3[:, :, 64].rearrange("p (k g j) -> p k g j",
                                                k=NBANK, g=G),
                    in1=sown_b, op=mybir.AluOpType.add)
                e_t = ed_p.tile([128, S], F32, tag="e")
                nc.scalar.activation(e_t[:], epre[:],
                                     mybir.ActivationFunctionType.Tanh)
                ex = ed_p.tile([128, S], F32, tag="ex")
                nc.scalar.activation(ex[:], e_t[:],
                                     mybir.ActivationFunctionType.Exp)
                exm = ed_p.tile([128, S], F32, tag="exm")
                nc.vector.tensor_mul(exm[:], ex[:], mask_sb[:, soff:soff + S])

                d1 = sm_p.tile([128, NBANK * G], F32, tag="d1")
                nc.vector.reduce_sum(
                    out=d1[:],
                    in_=exm[:].rearrange("p (kg j) -> p kg j", j=D),
                    axis=mybir.AxisListType.X)
                denom = sm_p.tile([128, G], F32, tag="denom")
                nc.vector.reduce_sum(
                    out=denom[:],
                    in_=d1[:].rearrange("p (k g) -> p g k", k=NBANK),
                    axis=mybir.AxisListType.X)
                rden = sm_p.tile([128, G], F32, tag="rden")
                nc.vector.reciprocal(rden[:], denom[:])
                att = ed_p.tile([128, S], F32, tag="att")
                rden_b = (rden[:].unsqueeze(1).broadcast_to([128, NBANK, G])
                          .unsqueeze(3).broadcast_to([128, NBANK, G, D]))
                nc.vector.tensor_tensor(
                    out=att[:].rearrange("p (k g j) -> p k g j", k=NBANK, g=G),
                    in0=exm[:].rearrange("p (k g j) -> p k g j", k=NBANK, g=G),
                    in1=rden_b, op=mybir.AluOpType.mult)

                w_t = w_p.tile([128, S * 64], F32, tag="w")
                nc.vector.tensor_tensor(
                    out=w_t[:].rearrange("p (s c) -> p s c", c=64),
                    in0=zg3[:, :, 0:64],
                    in1=att[:].unsqueeze(2).broadcast_to([128, S, 64]),
                    op=mybir.AluOpType.mult)
                r1 = sm_p.tile([128, NBANK * G * 64], F32, tag="r1")
                nc.vector.reduce_sum(
                    out=r1[:],
                    in_=w_t[:].rearrange("p (k g j c) -> p k g c j",
                                         k=NBANK, g=G, j=D),
                    axis=mybir.AxisListType.X)
                out_pre = sm_p.tile([128, G * 64], F32, tag="opre")
                nc.vector.reduce_sum(
                    out=out_pre[:],
                    in_=r1[:].rearrange("p (k g c) -> p g c k", k=NBANK, g=G),
                    axis=mybir.AxisListType.X)

                # normalize by unbiased std along features
                s1 = sm_p.tile([128, G], F32, tag="s1")
                nc.vector.reduce_sum(
                    out=s1[:],
                    in_=out_pre[:].rearrange("p (g c) -> p g c", c=64),
                    axis=mybir.AxisListType.X)
                sq = sm_p.tile([128, G * 64], F32, tag="sq")
                nc.vector.tensor_mul(sq[:], out_pre[:], out_pre[:])
                s2 = sm_p.tile([128, G], F32, tag="s2")
                nc.vector.reduce_sum(
                    out=s2[:],
                    in_=sq[:].rearrange("p (g c) -> p g c", c=64),
                    axis=mybir.AxisListType.X)
                m2 = sm_p.tile([128, G], F32, tag="m2")
                nc.vector.tensor_mul(m2[:], s1[:], s1[:])
                m2n = sm_p.tile([128, G], F32, tag="m2n")
                nc.vector.tensor_scalar_mul(m2n[:], m2[:], -1.0 / 64.0)
                var = sm_p.tile([128, G], F32, tag="var")
                nc.vector.tensor_add(var[:], m2n[:], s2[:])
                varn = sm_p.tile([128, G], F32, tag="varn")
                nc.vector.tensor_scalar_mul(varn[:], var[:], 1.0 / 63.0)
                rvar = sm_p.tile([128, G], F32, tag="rvar")
                nc.vector.reciprocal(rvar[:], varn[:])
                rstd = sm_p.tile([128, G], F32, tag="rstd")
                nc.scalar.activation(rstd[:], rvar[:],
                                     mybir.ActivationFunctionType.Sqrt)
                outn = sm_p.tile([128, G * 64], F32, tag="outn")
                nc.vector.tensor_tensor(
                    out=outn[:].rearrange("p (g c) -> p g c", c=64),
                    in0=out_pre[:].rearrange("p (g c) -> p g c", c=64),
                    in1=rstd[:].unsqueeze(2).broadcast_to([128, G, 64]),
                    op=mybir.AluOpType.mult)

                if layer == 1:
                    for g in range(G):
                        pt = pst_p.tile([D1, 128], F32, tag="pt")
                        nc.tensor.transpose(
                            out=pt[:], in_=outn[:, g * 64:(g + 1) * 64],
                            identity=ident[:])
                        nc.scalar.activation(
                            h1T_sb[0:D1, (t0 + g) * 128:(t0 + g + 1) * 128],
                            pt[:], mybir.ActivationFunctionType.Copy)
                else:
                    dr = out_my[:].rearrange("(t p) c -> p t c", p=128)
                    nc.sync.dma_start(
                        dr[:, t0:t0 + G, :],
                        outn[:].rearrange("p (g c) -> p g c", c=64))
                soff += S
                ioff += NBANK * (ni // 16)

        dense_layer(1, z1my, s_own1)
        if stage >= 1:
            nc.gpsimd.collective_compute(
                "AllGather", mybir.AluOpType.bypass,
                replica_groups=replica_groups, ins=[z1my[:]], outs=[z1[:]])
        if stage >= 2:
            edge_layer(z1, s_own1, layer=1)
        if stage >= 3:
            dense_layer(2, z2my, s_own2)
            nc.gpsimd.collective_compute(
                "AllGather", mybir.AluOpType.bypass,
                replica_groups=replica_groups, ins=[z2my[:]], outs=[z2[:]])
        if stage >= 4:
            edge_layer(z2, s_own2, layer=2)

    nc.compile()
    return nc


# ---------------------------------------------------------------------------
# public entry point
# ---------------------------------------------------------------------------

_CACHE = {}


def _get_program(plan, be1_half, be2_half):
    key = (be1_half, be2_half, plan["s_total"])
    if key not in _CACHE:
        _CACHE[key] = build_device_program(plan, be1_half, be2_half)
    return _CACHE[key]


def kernel(h, edge_index, W11, b11, W12, b12, W13, b13,
           W21, b21, W22, b22, W23, b23, _trace=False):
    plan = build_plan(edge_index)
    in_maps, be1_half, be2_half = host_inputs(
        h, plan, W11, b11, W12, b12, W13, b13, W21, b21, W22, b22, W23, b23)
    nc = _get_program(plan, be1_half, be2_half)
    res = run_bass_kernel_spmd(nc, in_maps, list(range(NCORES)), trace=_trace)
    out_new = np.concatenate([res.results[c]["out"] for c in range(NCORES)], 0)
    out = out_new[plan["new_of_old"]].astype(np.float32)
    kernel.last_results = res
    return out


kernel.last_results = None



# revision 3
# speedup vs baseline: 1.9966x; 1.9966x over previous
"""GAT 2-layer kernel for Trainium2, 8 NeuronCores.

Sharding: nodes are relabeled host-side so each core owns a contiguous block
of 12544 node ids with balanced edge counts (degree-sorted round-robin deal,
then an in-core sort by max-per-bank edge count to minimize gather padding).
Edges are partitioned by src row, so the per-row softmax and the output
accumulation are core-local.  Each core computes v = tanh(x@Wv.T+bv) and the
per-node scalar s = tanh(x@Wa.T+ba)@We.T for its own nodes into a [128]-wide
table row (v | s | pad), an AllGather shares the table, and the edge stage
fetches one 256B table row per edge with dma_gather (InstDMAGatherAnt).

dma_gather indices are int16, so the 100352-row table is gathered in 4 banks
of 25088 rows; each node's edge slots are grouped by dst bank.  Slot depths
are per-(tile-group, bank) (tile groups of <=2 tiles, S = sum_k G*D_k <= 128)
which keeps padding at ~1.57x the true edge count.  Gather chunks are issued
round-robin over 4 SWDGE queues (num_swdge_queues=4); with ~16 chunks in
flight the random 256B reads sustain ~110 GB/s (2.3 ns/desc) instead of the
~25 GB/s a single queue achieves.  Pad slots gather row 0 of the bank and are
zeroed by a host-built mask after exp.

Key algebraic simplification: (a[src]+a[dst]) @ We.T = s[src] + s[dst] with
s = a@We.T, so the edge MLP needs only per-node scalars.
"""

import numpy as np
from contextlib import ExitStack

import concourse.bass as bass
import concourse.tile as tile
from concourse import bacc, mybir
from concourse.bass_utils import run_bass_kernel_spmd

F32 = mybir.dt.float32
I16 = mybir.dt.int16
FP16 = mybir.dt.float16

NCORES = 8
N = 100_000
E = 1_600_000
D0 = 128
D1 = 64
NBANK = 4
ROWW = 128          # table row width (elements)
ZDT = FP16          # table dtype; 256B rows
S_MAX = 128
G_MAX = 2
NQ = 4              # SWDGE queues
CH = 8              # slots per gather chunk (1024 descs)


# ---------------------------------------------------------------------------
# host-side planning (pure index manipulation)
# ---------------------------------------------------------------------------

def build_plan(edge_index, n=N, ncores=NCORES):
    src = np.asarray(edge_index[0], dtype=np.int64)
    dst = np.asarray(edge_index[1], dtype=np.int64)
    deg = np.bincount(src, minlength=n).astype(np.int64)

    ranks_per_core = (n + ncores - 1) // ncores           # 12500
    tiles = (ranks_per_core + 127) // 128                  # 98
    npc = tiles * 128                                      # 12544
    npad = ncores * npc                                    # 100352
    bs = npad // NBANK                                     # 25088 (= 2 cores)

    # pass 1: deal nodes to cores by degree rank (balances edges per core);
    # the core then determines each node's bank (= core // 2).
    order = np.argsort(-deg, kind="stable")
    core_of_old = np.empty(n, dtype=np.int64)
    core_of_old[order] = np.arange(n) % ncores

    # per-node, per-bank out-edge counts (bank of an edge = dst's core // 2)
    bank_of_old_dst = core_of_old[dst] // 2
    cnt = np.zeros((n, NBANK), dtype=np.int64)
    np.add.at(cnt, (src, bank_of_old_dst), 1)
    maxb = cnt.max(1)

    # pass 2: within each core, order nodes by max-per-bank count desc
    new_of_old = np.empty(n, dtype=np.int64)
    for c in range(ncores):
        own = np.where(core_of_old == c)[0]
        own = own[np.argsort(-maxb[own], kind="stable")]
        new_of_old[own] = c * npc + np.arange(len(own))

    nsrc = new_of_old[src]
    ndst = new_of_old[dst]
    # group edges by (src, bank)
    ekey = nsrc * NBANK + (ndst // bs)
    eorder = np.argsort(ekey, kind="stable")
    ndst_sorted = ndst[eorder]
    ecnt = np.bincount(ekey, minlength=npad * NBANK).reshape(npad, NBANK)
    estart = np.zeros(npad * NBANK + 1, dtype=np.int64)
    np.cumsum(ecnt.reshape(-1), out=estart[1:])
    estart = estart[:-1].reshape(npad, NBANK)

    cnt_new = np.zeros((npad, NBANK), dtype=np.int64)
    cnt_new[new_of_old] = cnt
    # pad nodes (deg 0): one fake edge in bank 0 so softmax denom > 0
    padnode = cnt_new.sum(1) == 0
    cnt_new[padnode, 0] = 1
    # per-(tile, bank) depth, max over cores (SPMD: one program)
    c4 = cnt_new.reshape(ncores, tiles, 128, NBANK)
    dt_tile = c4.max(axis=(0, 2))                         # [tiles, NBANK]

    # greedy tile groups: G <= G_MAX tiles, S = G * sum_k max(D_k) <= S_MAX
    groups = []
    t = 0
    while t < tiles:
        ds = dt_tile[t].copy()
        g = 1
        while (t + g < tiles and g < G_MAX
               and (g + 1) * int(np.maximum(ds, dt_tile[t + g]).sum()) <= S_MAX):
            ds = np.maximum(ds, dt_tile[t + g])
            g += 1
        if g * int(ds.sum()) > S_MAX:
            # single tile exceeding S_MAX cannot happen for this graph
            # (max per-tile sum ~56), but guard anyway
            raise ValueError("tile exceeds S_MAX")
        groups.append((t, g, tuple(int(x) for x in ds)))
        t += g

    s_total = sum(g * sum(ds) for (_, g, ds) in groups)
    idxcols = 8 * s_total

    gidx = np.zeros((ncores, 128, idxcols), dtype=np.int16)
    maskh = np.zeros((ncores, 128, s_total), dtype=np.float32)
    jj64 = np.arange(64, dtype=np.int64)

    for ci in range(ncores):
        soff = 0
        ioff = 0
        for (t0, G, Ds) in groups:
            nids = (ci * npc + t0 * 128 + np.arange(G * 128)).reshape(G, 128)
            for k in range(NBANK):
                D = Ds[k]
                if D == 0:
                    continue
                ni = 128 * G * D
                st = estart[nids, k][:, :, None] + jj64[None, None, :D]
                cn = cnt_new[nids, k]
                valid = jj64[None, None, :D] < cn[:, :, None]
                vals = np.where(valid, ndst_sorted[np.minimum(st, E - 1)] - k * bs, 0)
                # fake edge for pad nodes: bank 0, idx 0 (valid already set)
                if k == 0:
                    pd = padnode[nids]
                    valid = valid | (pd[:, :, None] & (jj64[None, None, :D] < 1))
                    vals = np.where(pd[:, :, None], 0, vals)
                # slot i = (g*D + j)*128 + p  ->  idx16[16q + i%16, i//16]
                pos_val = vals.transpose(1, 0, 2).reshape(128, G * D)
                flat = pos_val.T.reshape(-1)                 # [(g j) p] -> i order
                wrapped = np.zeros((16, ni // 16), np.int16)
                ii = np.arange(ni)
                wrapped[ii % 16, ii // 16] = flat
                gidx[ci, :, ioff:ioff + ni // 16] = np.tile(wrapped, (8, 1))
                mk = valid.transpose(1, 0, 2).reshape(128, G * D).astype(np.float32)
                maskh[ci, :, soff:soff + G * D] = mk
                ioff += ni // 16
                soff += G * D

    return dict(
        new_of_old=new_of_old, npc=npc, npad=npad, tiles=tiles, bs=bs,
        groups=groups, s_total=s_total, idxcols=idxcols, gidx=gidx,
        maskh=maskh, ranks_per_core=ranks_per_core,
    )


def host_inputs(h, plan, W11, b11, W12, b12, W13, b13,
                W21, b21, W22, b22, W23, b23):
    npc, ncores = plan["npc"], NCORES
    new_of_old = plan["new_of_old"]
    h_new = np.zeros((plan["npad"], D0), dtype=np.float32)
    h_new[new_of_old] = np.asarray(h, dtype=np.float32)

    rep = np.ones((128, 1), dtype=np.float32)
    base = dict(
        waT1=np.ascontiguousarray(W11.T.astype(np.float32)),
        wvT1=np.ascontiguousarray(W13.T.astype(np.float32)),
        ba1r=rep * b11[None, :].astype(np.float32),
        bv1r=rep * b13[None, :].astype(np.float32),
        we1r=rep * W12[0][None, :].astype(np.float32),
        wa2e=np.concatenate([W21.T, b21[None, :]], 0).astype(np.float32),
        wv2e=np.concatenate([W23.T, b23[None, :]], 0).astype(np.float32),
        we2r=rep * W22[0][None, :].astype(np.float32),
        ident=np.eye(128, dtype=np.float32),
    )
    be1_half = float(b12[0]) * 0.5
    be2_half = float(b22[0]) * 0.5

    in_maps = []
    for c in range(ncores):
        m = dict(base)
        m["hT"] = np.ascontiguousarray(h_new[c * npc:(c + 1) * npc].T)
        m["gidx"] = np.ascontiguousarray(plan["gidx"][c])
        m["maskh"] = np.ascontiguousarray(plan["maskh"][c])
        in_maps.append(m)
    return in_maps, be1_half, be2_half


# ---------------------------------------------------------------------------
# device program (single SPMD program; per-core differences live in the data)
# ---------------------------------------------------------------------------

def build_device_program(plan, be1_half, be2_half, ncores=NCORES, stage=4):
    npc = plan["npc"]
    npad = plan["npad"]
    tiles = plan["tiles"]
    groups = plan["groups"]
    s_total = plan["s_total"]
    idxcols = plan["idxcols"]
    bs = plan["bs"]

    nc = bacc.Bacc("TRN2", target_bir_lowering=False, debug=False,
                   num_devices=ncores, dynamic_dma_scratch_size=32768,
                   num_swdge_queues=NQ)

    hT = nc.dram_tensor("hT", [D0, npc], F32, kind="ExternalInput")
    gidx_d = nc.dram_tensor("gidx", [128, idxcols], I16, kind="ExternalInput")
    mask_d = nc.dram_tensor("maskh", [128, s_total], F32, kind="ExternalInput")
    waT1 = nc.dram_tensor("waT1", [D0, D1], F32, kind="ExternalInput")
    wvT1 = nc.dram_tensor("wvT1", [D0, D1], F32, kind="ExternalInput")
    ba1r = nc.dram_tensor("ba1r", [128, D1], F32, kind="ExternalInput")
    bv1r = nc.dram_tensor("bv1r", [128, D1], F32, kind="ExternalInput")
    we1r = nc.dram_tensor("we1r", [128, D1], F32, kind="ExternalInput")
    wa2e = nc.dram_tensor("wa2e", [D1 + 1, D1], F32, kind="ExternalInput")
    wv2e = nc.dram_tensor("wv2e", [D1 + 1, D1], F32, kind="ExternalInput")
    we2r = nc.dram_tensor("we2r", [128, D1], F32, kind="ExternalInput")
    ident_d = nc.dram_tensor("ident", [128, 128], F32, kind="ExternalInput")
    out_my = nc.dram_tensor("out", [npc, D1], F32, kind="ExternalOutput")

    z1my = nc.dram_tensor("z1my", [npc, ROWW], ZDT)
    z2my = nc.dram_tensor("z2my", [npc, ROWW], ZDT)
    z1 = nc.dram_tensor("z1", [npad, ROWW], ZDT, addr_space="Shared")
    z2 = nc.dram_tensor("z2", [npad, ROWW], ZDT, addr_space="Shared")
    replica_groups = [list(range(ncores))]

    rr = [0]  # SWDGE queue round-robin

    with tile.TileContext(nc) as tc, ExitStack() as ctx:
        const_p = ctx.enter_context(tc.tile_pool(name="const", bufs=1))
        xt_p = ctx.enter_context(tc.tile_pool(name="xt", bufs=2))
        ps_p = ctx.enter_context(tc.tile_pool(name="ps", bufs=2, space="PSUM"))
        pst_p = ctx.enter_context(tc.tile_pool(name="pst", bufs=2, space="PSUM"))
        dense_p = ctx.enter_context(tc.tile_pool(name="dense", bufs=2))
        zrow_p = ctx.enter_context(tc.tile_pool(name="zrow", bufs=2))
        idx_p = ctx.enter_context(tc.tile_pool(name="idx", bufs=3))
        zg_p = ctx.enter_context(tc.tile_pool(name="zg", bufs=2))
        w_p = ctx.enter_context(tc.tile_pool(name="w", bufs=2))
        ed_p = ctx.enter_context(tc.tile_pool(name="ed", bufs=2))
        sm_p = ctx.enter_context(tc.tile_pool(name="sm", bufs=2))

        def ld_const(dram, shape, tag, dt=F32):
            t = const_p.tile(shape, dt, tag=tag)
            nc.sync.dma_start(t[:], dram[:])
            return t

        waT1_sb = ld_const(waT1, [D0, D1], "waT1")
        wvT1_sb = ld_const(wvT1, [D0, D1], "wvT1")
        ba1_sb = ld_const(ba1r, [128, D1], "ba1")
        bv1_sb = ld_const(bv1r, [128, D1], "bv1")
        we1_sb = ld_const(we1r, [128, D1], "we1")
        wa2_sb = ld_const(wa2e, [D1 + 1, D1], "wa2")
        wv2_sb = ld_const(wv2e, [D1 + 1, D1], "wv2")
        we2_sb = ld_const(we2r, [128, D1], "we2")
        mask_sb = ld_const(mask_d, [128, s_total], "mask")
        ident = ld_const(ident_d, [128, 128], "ident")

        s_own1 = const_p.tile([128, tiles], F32)
        s_own2 = const_p.tile([128, tiles], F32)
        h1T_sb = const_p.tile([D1 + 1, npc], F32)
        nc.vector.memset(h1T_sb[D1:D1 + 1, :], 1.0)

        # ----------------- dense stage -----------------
        def dense_layer(layer, zmy, s_own):
            for bt in range(0, tiles, 8):
                nb = min(8, tiles - bt)
                zrow = zrow_p.tile([128, 8 * ROWW], ZDT, tag="zrow")
                nc.vector.memset(zrow[:], 0)
                zr = zrow[:].rearrange("p (k c) -> p k c", c=ROWW)
                for k in range(nb):
                    t = bt + k
                    if layer == 1:
                        xt = xt_p.tile([D0, 128], F32)
                        nc.sync.dma_start(xt[:], hT[:, t * 128:(t + 1) * 128])
                        lhs, wa, wv = xt[:], waT1_sb[:], wvT1_sb[:]
                    else:
                        lhs, wa, wv = (h1T_sb[:, t * 128:(t + 1) * 128],
                                       wa2_sb[:], wv2_sb[:])
                    pa = ps_p.tile([128, D1], F32, tag="pa")
                    nc.tensor.matmul(pa[:], lhsT=lhs, rhs=wa, start=True, stop=True)
                    pv = ps_p.tile([128, D1], F32, tag="pv")
                    nc.tensor.matmul(pv[:], lhsT=lhs, rhs=wv, start=True, stop=True)
                    a_sb = dense_p.tile([128, D1], F32, tag="a")
                    if layer == 1:
                        apre = dense_p.tile([128, D1], F32, tag="apre")
                        nc.vector.tensor_add(apre[:], pa[:], ba1_sb[:])
                        nc.scalar.activation(a_sb[:], apre[:],
                                             mybir.ActivationFunctionType.Tanh)
                        vpre = dense_p.tile([128, D1], F32, tag="vpre")
                        nc.vector.tensor_add(vpre[:], pv[:], bv1_sb[:])
                        nc.scalar.activation(zr[:, k, 0:64], vpre[:],
                                             mybir.ActivationFunctionType.Tanh)
                    else:
                        nc.scalar.activation(a_sb[:], pa[:],
                                             mybir.ActivationFunctionType.Tanh)
                        nc.scalar.activation(zr[:, k, 0:64], pv[:],
                                             mybir.ActivationFunctionType.Tanh)
                    scr = dense_p.tile([128, D1], F32, tag="scr")
                    nc.vector.tensor_mul(
                        scr[:], a_sb[:],
                        we1_sb[:] if layer == 1 else we2_sb[:])
                    rs = dense_p.tile([128, 1], F32, tag="rs")
                    nc.vector.reduce_sum(out=rs[:], in_=scr[:],
                                         axis=mybir.AxisListType.X)
                    nc.vector.tensor_scalar_add(
                        s_own[:, t:t + 1], rs[:],
                        be1_half if layer == 1 else be2_half)
                    nc.vector.tensor_copy(zr[:, k, 64:65], s_own[:, t:t + 1])
                dr = zmy[:].rearrange("(t p) c -> p t c", p=128)
                nc.sync.dma_start(dr[:, bt:bt + nb, :], zr[:, 0:nb, :])

        # ----------------- edge stage -----------------
        def edge_layer(z_full, s_own, layer):
            soff = 0
            ioff = 0
            for (t0, G, Ds) in groups:
                S = G * sum(Ds)
                it = idx_p.tile([128, 8 * S], I16, tag="it")
                nc.sync.dma_start(it[:], gidx_d[:, ioff:ioff + 8 * S])
                zg = zg_p.tile([128, S * ROWW], ZDT, tag="zg")
                zg3 = zg[:].rearrange("p (s c) -> p s c", c=ROWW)
                off = 0
                icol = 0
                for k in range(NBANK):
                    GD = G * Ds[k]
                    if GD == 0:
                        continue
                    for j0 in range(0, GD, CH):
                        js = min(CH, GD - j0)
                        nj = 128 * js
                        nc.gpsimd.dma_gather(
                            out_ap=zg3[:, off + j0:off + j0 + js, :],
                            in_ap=z_full[k * bs:(k + 1) * bs, :],
                            idxs_ap=it[:, icol + j0 * 8:icol + (j0 + js) * 8],
                            num_idxs=nj, num_idxs_reg=nj, elem_size=ROWW,
                            queue_num=rr[0] % NQ)
                        rr[0] += 1
                    off += GD
                    icol += 8 * GD

                # epre = s_dst + s_src (per-bank windows; fp16)
                epre = ed_p.tile([128, S], FP16, tag="epre")
                off = 0
                for k in range(NBANK):
                    GD = G * Ds[k]
                    if GD == 0:
                        continue
                    sown_b = (s_own[:, t0:t0 + G].unsqueeze(2)
                              .broadcast_to([128, G, Ds[k]]))
                    nc.vector.tensor_tensor(
                        out=epre[:, off:off + GD].rearrange(
                            "p (g j) -> p g j", g=G),
                        in0=zg3[:, off:off + GD, 64].rearrange(
                            "p (g j) -> p g j", g=G),
                        in1=sown_b, op=mybir.AluOpType.add)
                    off += GD
                e_t = ed_p.tile([128, S], FP16, tag="e")
                nc.scalar.activation(e_t[:], epre[:],
                                     mybir.ActivationFunctionType.Tanh)
                ex = ed_p.tile([128, S], FP16, tag="ex")
                nc.scalar.activation(ex[:], e_t[:],
                                     mybir.ActivationFunctionType.Exp)
                exm = ed_p.tile([128, S], FP16, tag="exm")
                nc.vector.tensor_mul(exm[:], ex[:], mask_sb[:, soff:soff + S])

                # denominator: per-bank windowed reduce, accumulate
                denom = sm_p.tile([128, G], F32, tag="denom")
                dpart = sm_p.tile([128, G], F32, tag="dpart")
                off = 0
                first = True
                for k in range(NBANK):
                    GD = G * Ds[k]
                    if GD == 0:
                        continue
                    tgt = denom if first else dpart
                    nc.vector.reduce_sum(
                        out=tgt[:],
                        in_=exm[:, off:off + GD].rearrange(
                            "p (g j) -> p g j", g=G),
                        axis=mybir.AxisListType.X)
                    if not first:
                        nc.vector.tensor_add(denom[:], denom[:], dpart[:])
                    first = False
                    off += GD
                rden = sm_p.tile([128, G], F32, tag="rden")
                nc.vector.reciprocal(rden[:], denom[:])

                # att = exm * (1/denom), fp16
                att = ed_p.tile([128, S], FP16, tag="att")
                off = 0
                for k in range(NBANK):
                    GD = G * Ds[k]
                    if GD == 0:
                        continue
                    rden_b = (rden[:].unsqueeze(2)
                              .broadcast_to([128, G, Ds[k]]))
                    nc.vector.tensor_tensor(
                        out=att[:, off:off + GD].rearrange(
                            "p (g j) -> p g j", g=G),
                        in0=exm[:, off:off + GD].rearrange(
                            "p (g j) -> p g j", g=G),
                        in1=rden_b, op=mybir.AluOpType.mult)
                    off += GD

                # weighted values: w = v * att, fp16; then windowed reduce
                w_t = w_p.tile([128, S * 64], FP16, tag="w")
                nc.vector.tensor_tensor(
                    out=w_t[:].rearrange("p (s c) -> p s c", c=64),
                    in0=zg3[:, :, 0:64],
                    in1=att[:].unsqueeze(2).broadcast_to([128, S, 64]),
                    op=mybir.AluOpType.mult)
                out_pre = sm_p.tile([128, G * 64], F32, tag="opre")
                rpart = sm_p.tile([128, G * 64], F32, tag="rpart")
                off = 0
                first = True
                for k in range(NBANK):
                    GD = G * Ds[k]
                    if GD == 0:
                        continue
                    tgt = out_pre if first else rpart
                    nc.vector.reduce_sum(
                        out=tgt[:],
                        in_=w_t[:, off * 64:(off + GD) * 64].rearrange(
                            "p (g j c) -> p g c j", g=G, j=Ds[k]),
                        axis=mybir.AxisListType.X)
                    if not first:
                        nc.vector.tensor_add(out_pre[:], out_pre[:], rpart[:])
                    first = False
                    off += GD

                # normalize by unbiased std along features
                s1 = sm_p.tile([128, G], F32, tag="s1")
                nc.vector.reduce_sum(
                    out=s1[:],
                    in_=out_pre[:].rearrange("p (g c) -> p g c", c=64),
                    axis=mybir.AxisListType.X)
                sq = sm_p.tile([128, G * 64], F32, tag="sq")
                nc.vector.tensor_mul(sq[:], out_pre[:], out_pre[:])
                s2 = sm_p.tile([128, G], F32, tag="s2")
                nc.vector.reduce_sum(
                    out=s2[:],
                    in_=sq[:].rearrange("p (g c) -> p g c", c=64),
                    axis=mybir.AxisListType.X)
                m2 = sm_p.tile([128, G], F32, tag="m2")
                nc.vector.tensor_mul(m2[:], s1[:], s1[:])
                m2n = sm_p.tile([128, G], F32, tag="m2n")
                nc.vector.tensor_scalar_mul(m2n[:], m2[:], -1.0 / 64.0)
                var = sm_p.tile([128, G], F32, tag="var")
                nc.vector.tensor_add(var[:], m2n[:], s2[:])
                varn = sm_p.tile([128, G], F32, tag="varn")
                nc.vector.tensor_scalar_mul(varn[:], var[:], 1.0 / 63.0)
                rvar = sm_p.tile([128, G], F32, tag="rvar")
                nc.vector.reciprocal(rvar[:], varn[:])
                rstd = sm_p.tile([128, G], F32, tag="rstd")
                nc.scalar.activation(rstd[:], rvar[:],
                                     mybir.ActivationFunctionType.Sqrt)
                outn = sm_p.tile([128, G * 64], F32, tag="outn")
                nc.vector.tensor_tensor(
                    out=outn[:].rearrange("p (g c) -> p g c", c=64),
                    in0=out_pre[:].rearrange("p (g c) -> p g c", c=64),
                    in1=rstd[:].unsqueeze(2).broadcast_to([128, G, 64]),
                    op=mybir.AluOpType.mult)

                if layer == 1:
                    for g in range(G):
                        pt = pst_p.tile([D1, 128], F32, tag="pt")
                        nc.tensor.transpose(
                            out=pt[:], in_=outn[:, g * 64:(g + 1) * 64],
                            identity=ident[:])
                        nc.scalar.activation(
                            h1T_sb[0:D1, (t0 + g) * 128:(t0 + g + 1) * 128],
                            pt[:], mybir.ActivationFunctionType.Copy)
                else:
                    dr = out_my[:].rearrange("(t p) c -> p t c", p=128)
                    nc.sync.dma_start(
                        dr[:, t0:t0 + G, :],
                        outn[:].rearrange("p (g c) -> p g c", c=64))
                soff += S
                ioff += 8 * S

        dense_layer(1, z1my, s_own1)
        if stage >= 1:
            nc.gpsimd.collective_compute(
                "AllGather", mybir.AluOpType.bypass,
                replica_groups=replica_groups, ins=[z1my[:]], outs=[z1[:]])
        if stage >= 2:
            edge_layer(z1, s_own1, layer=1)
        if stage >= 3:
            dense_layer(2, z2my, s_own2)
            nc.gpsimd.collective_compute(
                "AllGather", mybir.AluOpType.bypass,
                replica_groups=replica_groups, ins=[z2my[:]], outs=[z2[:]])
        if stage >= 4:
            edge_layer(z2, s_own2, layer=2)

    nc.compile()
    return nc


# ---------------------------------------------------------------------------
# public entry point
# ---------------------------------------------------------------------------

_CACHE = {}


def _get_program(plan, be1_half, be2_half):
    key = (be1_half, be2_half, plan["s_total"])
    if key not in _CACHE:
        _CACHE[key] = build_device_program(plan, be1_half, be2_half)
    return _CACHE[key]


def kernel(h, edge_index, W11, b11, W12, b12, W13, b13,
           W21, b21, W22, b22, W23, b23, _trace=False):
    plan = build_plan(edge_index)
    in_maps, be1_half, be2_half = host_inputs(
        h, plan, W11, b11, W12, b12, W13, b13, W21, b21, W22, b22, W23, b23)
    nc = _get_program(plan, be1_half, be2_half)
    res = run_bass_kernel_spmd(nc, in_maps, list(range(NCORES)), trace=_trace)
    out_new = np.concatenate([res.results[c]["out"] for c in range(NCORES)], 0)
    out = out_new[plan["new_of_old"]].astype(np.float32)
    kernel.last_results = res
    return out


kernel.last_results = None
